# revision 20
# baseline (speedup 1.0000x reference)
"""Trainium2 Bass kernel for nn_DecoderAttention (B=32, LQ=256, LK=2048, D=512, H=8).

Data-parallel over batch across 8 NeuronCores (4 batch items each).
All matmuls run in bf16 (1 col/cycle at warm 2.4GHz PE clock).

Per batch item (transposed-side layout, contraction always on partitions):
  k loaded via gpsimd cast-DMA (f32 DRAM -> bf16 SBUF), PE-transposed to
  kT[d, l] in l-groups of 512; kp/k2/vp projections stream per group.
  Per head-pair t_: S^T[l, q] for both heads into one [128,512] PSUM bank
  (cols 0:256 / 256:512) -> ONE Exp activation (mask bias per l-partition)
  -> E bf16; U = [v | 1]^T E accumulates over all 16 l-blocks in one PSUM
  bank (even head rows 0:65 cols 0:256 with Z last; odd head rows 63:128
  cols 256:512 with Z first, so ctx rows land lane-aligned at 0:64/64:128).
  Z broadcast via ones-matmul, one reciprocal, two lane-aligned muls,
  + bv as per-partition bias -> ctx pair [128, 256] bf16.
  out_proj = 4-step accumulation over head-pairs; final scores = ncT^T@k2T,
  tanh*CLIP (ACT tanh + DVE mul), mask fill via copy_predicated, DMA out.

Cross-batch overlap via bufs=2 tile pools keeps the PE HAM-warm.
"""
import sys

sys.path.insert(0, "/opt/trn_rl_repo")

import numpy as np

import concourse.bass as bass
import concourse.bacc as bacc
import concourse.mybir as mybir
import concourse.tile as tile
from concourse import bass_utils
from concourse.masks import make_identity

F32 = mybir.dt.float32
BF16 = mybir.dt.bfloat16
U8 = mybir.dt.uint8
AF = mybir.ActivationFunctionType

B, LQ, LK, D, H = 32, 256, 2048, 512, 8
HD = D // H              # 64
NCORES = 8
BPC = B // NCORES        # 4 batch items per core
NLB = LK // 128          # 16 l-blocks
NG = LK // 512           # 4 l-groups
CLIP = 10.0
FLOAT_MIN = -3.4e38
ISQ_HD = 0.125           # 1/sqrt(64)
ISQ_D = float(1.0 / np.sqrt(512.0))
MASK_BIG = -1e30
W_NAMES = ("Wq", "Wk", "Wv", "Wks", "Wo")
B_OF_W = {"Wq": "bq", "Wk": "bk", "Wv": "bv", "Wo": "bo", "Wks": "bks"}

TRACE = False
LAST_RESULTS = None
_CACHE = {}


def _build(reps=1):
    nc = bacc.Bacc("TRN2", target_bir_lowering=False, debug=False)
    q_d = nc.dram_tensor("q", [BPC, LQ, D], F32, kind="ExternalInput").ap()
    k_d = nc.dram_tensor("k", [BPC, LK, D], F32, kind="ExternalInput").ap()
    m_d = nc.dram_tensor("mask", [BPC, LK], U8, kind="ExternalInput").ap()
    w_d = {n: nc.dram_tensor(n, [D, D], F32, kind="ExternalInput").ap()
           for n in W_NAMES}
    b_d = {n: nc.dram_tensor(B_OF_W[n], [D], F32, kind="ExternalInput").ap()
           for n in W_NAMES}
    out_d = nc.dram_tensor("out", [BPC, LQ, LK], F32, kind="ExternalOutput").ap()

    lowp = nc.allow_low_precision("bf16 matmul operands by design")
    lowp.__enter__()
    with tile.TileContext(nc) as tc:
        with (
            tc.tile_pool(name="c1", bufs=1) as c1,          # persistent consts
            tc.tile_pool(name="pb", bufs=2) as pb,          # per-batch persistents
            tc.tile_pool(name="vpap", bufs=24) as vpap,
            tc.tile_pool(name="knp", bufs=5) as knp,        # k natural staging
            tc.tile_pool(name="ktp", bufs=8) as ktp,        # kT group tiles
            tc.tile_pool(name="etp", bufs=3) as etp,        # exp output tiles
            tc.tile_pool(name="thp", bufs=2) as thp,        # final output staging
            tc.tile_pool(name="smal", bufs=2) as smal,      # small working tiles
            tc.tile_pool(name="tr_ps", bufs=1, space="PSUM") as tr_ps,
            tc.tile_pool(name="mm_ps", bufs=2, space="PSUM") as mm_ps,
            tc.tile_pool(name="st_ps", bufs=3, space="PSUM") as st_ps,
            tc.tile_pool(name="u_ps", bufs=2, space="PSUM") as u_ps,
        ):
            # ---------------- one-time setup ----------------
            identf = knp.tile([128, 128], F32, tag="identf", name="identf")
            make_identity(nc, identf)
            ident = c1.tile([128, 128], BF16, tag="ident", name="ident")
            nc.vector.tensor_copy(ident, identf)
            onr = c1.tile([128, 64], BF16, tag="onr", name="onr")
            nc.vector.memset(onr, 1.0)
            fmin = c1.tile([128, 1], F32, tag="fmin", name="fmin")
            nc.vector.memset(fmin, FLOAT_MIN)

            # biases as [128,1] column chunks (f32)
            bcol = {}
            for wn in W_NAMES:
                bn = B_OF_W[wn]
                ap3 = b_d[wn].rearrange("(c p one) -> c p one", p=128, one=1)
                for c in range(4):
                    t = c1.tile([128, 1], F32, tag=f"b_{bn}_{c}", name=f"b_{bn}_{c}")
                    nc.sync.dma_start(out=t, in_=ap3[c])
                    bcol[(bn, c)] = t
            # bv per head [64,1]
            bvh = []
            aph = b_d["Wv"].rearrange("(h p one) -> h p one", p=64, one=1)
            for h in range(H):
                t = c1.tile([64, 1], F32, tag=f"bvh{h}", name=f"bvh{h}")
                nc.sync.dma_start(out=t, in_=aph[h])
                bvh.append(t)

            # transposed weights WT[(wn, dk)] = [128(din chunk), 512(dout)] bf16
            WT = {}
            for wn in ("Wq", "Wk", "Wv", "Wks"):
                wnat = []
                for nj in range(4):
                    t = knp.tile([128, 512], BF16, tag="knat", name="knat")
                    nc.gpsimd.dma_start(
                        out=t, in_=w_d[wn][nj * 128:(nj + 1) * 128, :])
                    wnat.append(t)
                for dk in range(4):
                    pt = tr_ps.tile([128, 512], BF16, tag="tr", name="tr")
                    for nj in range(4):
                        nc.tensor.transpose(
                            pt[:, nj * 128:(nj + 1) * 128],
                            wnat[nj][:, dk * 128:(dk + 1) * 128], ident)
                    wt = c1.tile([128, 512], BF16, tag=f"wt_{wn}_{dk}",
                                 name=f"wt_{wn}_{dk}")
                    nc.any.tensor_copy(wt, pt)
                    WT[(wn, dk)] = wt
            # Wo per head: WoTh[h] = [64(din in head), 512(dout)] bf16, base 0
            WoTh = []
            wnat = []
            for nj in range(4):
                t = knp.tile([128, 512], BF16, tag="knat", name="knat")
                nc.gpsimd.dma_start(
                    out=t, in_=w_d["Wo"][nj * 128:(nj + 1) * 128, :])
                wnat.append(t)
            for h in range(H):
                pt = tr_ps.tile([128, 512], BF16, tag="tr", name="tr")
                for nj in range(4):
                    nc.tensor.transpose(
                        pt[0:64, nj * 128:(nj + 1) * 128],
                        wnat[nj][:, h * 64:(h + 1) * 64], ident)
                wt = c1.tile([64, 512], BF16, tag=f"woth{h}", name=f"woth{h}")
                nc.any.tensor_copy(wt, pt[0:64, :])
                WoTh.append(wt)

            # ---------------- per batch item ----------------
            for bi in [b for _ in range(reps) for b in range(BPC)]:
                # mask: broadcast row to all partitions (uint8) for final fill
                m_row = m_d[bi]
                bcast = bass.AP(tensor=m_row.tensor, offset=m_row.offset,
                                ap=[[0, 128]] + m_row.ap)
                masku8 = pb.tile([128, LK], U8, tag="masku8", name="masku8")
                nc.gpsimd.dma_start(out=masku8, in_=bcast)
                # mask bias columns mb[:, lcg] = -1e30 * mask[lcg*128 + p]
                m16 = smal.tile([16, 128], U8, tag="m16", name="m16")
                nc.sync.dma_start(
                    out=m16, in_=m_row.rearrange("(c p) -> c p", c=16))
                m16f = smal.tile([16, 128], BF16, tag="m16f", name="m16f")
                nc.vector.tensor_copy(m16f, m16)
                mpt = tr_ps.tile([128, 512], BF16, tag="tr", name="tr")
                nc.tensor.transpose(mpt[:, 0:16], m16f, ident[0:16, 0:16])
                mb = pb.tile([128, 16], F32, tag="mb", name="mb")
                nc.vector.tensor_scalar_mul(mb, mpt[:, 0:16], MASK_BIG)

                # q: cast-load + transpose + project
                qnat = []
                for mi in range(2):
                    t = knp.tile([128, 512], BF16, tag="knat", name="knat")
                    nc.gpsimd.dma_start(
                        out=t, in_=q_d[bi, mi * 128:(mi + 1) * 128, :])
                    qnat.append(t)
                qTr = []
                for dk in range(4):
                    pt = tr_ps.tile([128, 512], BF16, tag="tr", name="tr")
                    for mi in range(2):
                        nc.tensor.transpose(
                            pt[:, mi * 128:(mi + 1) * 128],
                            qnat[mi][:, dk * 128:(dk + 1) * 128], ident)
                    t = c1.tile([128, 256], BF16, tag=f"qtr{dk}", name=f"qtr{dk}")
                    nc.any.tensor_copy(t, pt[:, 0:256])
                    qTr.append(t)
                qpTr = []
                for nj in range(4):
                    pt = mm_ps.tile([128, 512], F32, tag="mm", name="mm")
                    for dk in range(4):
                        nc.tensor.matmul(
                            pt[:, 0:256], WT[("Wq", dk)][:, nj * 128:(nj + 1) * 128],
                            qTr[dk], start=(dk == 0), stop=(dk == 3))
                    t = pb.tile([128, 256], BF16, tag=f"qptr{nj}", name=f"qptr{nj}")
                    nc.scalar.activation(t, pt[:, 0:256], AF.Identity,
                                         bias=bcol[("bq", nj)][:, :], scale=1.0)
                    qpTr.append(t)
                qpOd = []
                for nj in range(4):
                    t = pb.tile([64, 256], BF16, tag=f"qpo{nj}", name=f"qpo{nj}")
                    nc.sync.dma_start(out=t, in_=qpTr[nj][64:128, :])
                    qpOd.append(t)

                kpTr = [pb.tile([128, LK], BF16, tag=f"kpt{nj}", name=f"kpt{nj}")
                        for nj in range(4)]
                # odd-head rows moved to base partition 0 (lane shift via DMA)
                # so every attention matmul operand sits at base 0.
                kpOd = [pb.tile([64, LK], BF16, tag=f"kpo{nj}", name=f"kpo{nj}")
                        for nj in range(4)]
                k2Tr = [pb.tile([128, LK], BF16, tag=f"k2t{nj}", name=f"k2t{nj}")
                        for nj in range(4)]
                vpa = []

                for g in range(NG):
                    lbase = g * 512
                    knat = []
                    for li in range(4):
                        t = knp.tile([128, 512], BF16, tag="knat", name="knat")
                        nc.gpsimd.dma_start(
                            out=t, in_=k_d[bi, lbase + li * 128:lbase + (li + 1) * 128, :])
                        knat.append(t)
                    kTg = []
                    for dk in range(4):
                        pt = tr_ps.tile([128, 512], BF16, tag="tr", name="tr")
                        for li in range(4):
                            nc.tensor.transpose(
                                pt[:, li * 128:(li + 1) * 128],
                                knat[li][:, dk * 128:(dk + 1) * 128], ident)
                        t = ktp.tile([128, 512], BF16, tag="ktg", name="ktg")
                        if dk % 2 == 0:
                            nc.vector.tensor_copy(t, pt)
                        else:
                            nc.scalar.activation(t, pt, AF.Identity,
                                                 bias=0.0, scale=1.0)
                        kTg.append(t)
                    # kp projection (bias via DVE), k2 (bias via ACT)
                    for nj in range(4):
                        pt = mm_ps.tile([128, 512], F32, tag="mm", name="mm")
                        for dk in range(4):
                            nc.tensor.matmul(
                                pt, WT[("Wk", dk)][:, nj * 128:(nj + 1) * 128],
                                kTg[dk], start=(dk == 0), stop=(dk == 3))
                        nc.vector.tensor_scalar_add(
                            out=kpTr[nj][:, lbase:lbase + 512],
                            in0=pt, scalar1=bcol[("bk", nj)][:, :])
                        nc.sync.dma_start(
                            out=kpOd[nj][:, lbase:lbase + 512],
                            in_=kpTr[nj][64:128, lbase:lbase + 512])
                    for nj in range(4):
                        pt = mm_ps.tile([128, 512], F32, tag="mm", name="mm")
                        for dk in range(4):
                            nc.tensor.matmul(
                                pt, WT[("Wks", dk)][:, nj * 128:(nj + 1) * 128],
                                kTg[dk], start=(dk == 0), stop=(dk == 3))
                        nc.scalar.activation(
                            k2Tr[nj][:, lbase:lbase + 512], pt, AF.Identity,
                            bias=bcol[("bks", nj)][:, :], scale=1.0)
                    # vp natural [l, dout] -> vpa per head: [v(64) | one]
                    for lb in range(4):
                        pt = mm_ps.tile([128, 512], F32, tag="mm", name="mm")
                        for dk in range(4):
                            nc.tensor.matmul(
                                pt, kTg[dk][:, lb * 128:(lb + 1) * 128],
                                WT[("Wv", dk)], start=(dk == 0), stop=(dk == 3))
                        vt = vpap.tile([128, H * 65], BF16, tag="vpa", name="vpa")
                        vt3 = vt.rearrange("p (h c) -> p h c", c=65)
                        nc.vector.tensor_copy(
                            vt3[:, :, 0:64],
                            pt.rearrange("p (h c) -> p h c", c=64))
                        nc.vector.memset(vt3[:, :, 64:65], 1.0)
                        vpa.append(vt)

                # attention per head pair
                ctxh = []
                for t_ in range(4):
                    u = u_ps.tile([128, 512], F32, tag="u", name="u")
                    for lc in range(NLB):
                        sp = st_ps.tile([128, 512], F32, tag="st", name="st")
                        # hh=1 start=False: start clears the whole bank
                        nc.tensor.matmul(
                            sp[:, 0:256],
                            kpTr[t_][0:64, lc * 128:(lc + 1) * 128],
                            qpTr[t_][0:64, :],
                            start=True, stop=True, skip_group_check=True)
                        nc.tensor.matmul(
                            sp[:, 256:512],
                            kpOd[t_][:, lc * 128:(lc + 1) * 128],
                            qpOd[t_],
                            start=False, stop=True, skip_group_check=True)
                        et = etp.tile([128, 512], BF16, tag="et", name="et")
                        nc.scalar.activation(
                            et, sp, AF.Exp,
                            bias=mb[:, lc:lc + 1], scale=ISQ_HD)
                        nc.tensor.matmul(
                            u[0:65, 0:256],
                            vpa[lc][:, (2 * t_) * 65:(2 * t_) * 65 + 65],
                            et[:, 0:256], start=(lc == 0), stop=(lc == NLB - 1),
                            skip_group_check=True)
                        # start only on the bank's first matmul: start=True
                        # clears the WHOLE bank; the odd head's first matmul
                        # relies on has_written=0 -> overwrite semantics.
                        nc.tensor.matmul(
                            u[0:65, 256:512],
                            vpa[lc][:, (2 * t_ + 1) * 65:(2 * t_ + 1) * 65 + 65],
                            et[:, 256:512], start=False, stop=(lc == NLB - 1),
                            skip_group_check=True)
                    # Z row -> bf16, broadcast via ones-matmul, divide, + bv
                    zr = smal.tile([128, 512], BF16, tag="zr", name="zr")
                    nc.vector.tensor_copy(zr[64:65, :], u[64:65, :])
                    zb = mm_ps.tile([128, 512], F32, tag="mm", name="mm")
                    nc.tensor.matmul(zb[0:64, :], onr[64:65, :],
                                     zr[64:65, :], start=True, stop=True)
                    rz = smal.tile([64, 512], F32, tag="rz", name="rz")
                    nc.vector.reciprocal(rz, zb[0:64, :])
                    ct = c1.tile([64, 512], BF16, tag=f"ctx{t_}", name=f"ctx{t_}")
                    nc.vector.tensor_mul(ct, u[0:64, :], rz)
                    nc.vector.tensor_scalar_add(
                        out=ct[:, 0:256], in0=ct[:, 0:256],
                        scalar1=bvh[2 * t_][:, :])
                    nc.vector.tensor_scalar_add(
                        out=ct[:, 256:512], in0=ct[:, 256:512],
                        scalar1=bvh[2 * t_ + 1][:, :])
                    ctxh.append(ct)

                # out_proj: ncT[dout, q] = sum over heads (p=64 each)
                ncTr = []
                for nj in range(4):
                    pt = mm_ps.tile([128, 512], F32, tag="mm", name="mm")
                    for t_ in range(4):
                        for hh in range(2):
                            nc.tensor.matmul(
                                pt[:, 0:256],
                                WoTh[2 * t_ + hh][:, nj * 128:(nj + 1) * 128],
                                ctxh[t_][:, hh * 256:(hh + 1) * 256],
                                start=(t_ == 0 and hh == 0),
                                stop=(t_ == 3 and hh == 1))
                    t = c1.tile([128, 256], BF16, tag=f"nct{nj}", name=f"nct{nj}")
                    nc.scalar.activation(t, pt[:, 0:256], AF.Identity,
                                         bias=bcol[("bo", nj)][:, :], scale=1.0)
                    ncTr.append(t)

                # final scores + tanh clip + mask fill
                for mi in range(2):
                    for lg in range(4):
                        pt = mm_ps.tile([128, 512], F32, tag="mm", name="mm")
                        for nk in range(4):
                            nc.tensor.matmul(
                                pt, ncTr[nk][:, mi * 128:(mi + 1) * 128],
                                k2Tr[nk][:, lg * 512:(lg + 1) * 512],
                                start=(nk == 0), stop=(nk == 3))
                        th = thp.tile([128, 512], F32, tag="th", name="th")
                        nc.scalar.activation(th, pt, AF.Tanh,
                                             bias=0.0, scale=ISQ_D)
                        nc.vector.tensor_scalar_mul(th, th, CLIP)
                        nc.vector.copy_predicated(
                            th, masku8[:, lg * 512:(lg + 1) * 512],
                            fmin.to_broadcast([128, 512]))
                        nc.sync.dma_start(
                            out=out_d[bi, mi * 128:(mi + 1) * 128,
                                      lg * 512:(lg + 1) * 512],
                            in_=th)
    lowp.__exit__(None, None, None)
    nc.finalize()
    return nc


def kernel(**inputs):
    global LAST_RESULTS
    import os
    reps = int(os.environ.get("KERNEL_REPS", "1"))
    key = ("nc", reps)
    if key not in _CACHE:
        _CACHE[key] = _build(reps)
    nc = _CACHE[key]

    q = np.ascontiguousarray(np.asarray(inputs["q"], dtype=np.float32))
    k = np.ascontiguousarray(np.asarray(inputs["k"], dtype=np.float32))
    mask = np.ascontiguousarray(np.asarray(inputs["mask"]).astype(np.uint8))
    ws = {n: np.ascontiguousarray(np.asarray(inputs[n], dtype=np.float32))
          for n in W_NAMES}
    bs = {B_OF_W[n]: np.ascontiguousarray(
        np.asarray(inputs[B_OF_W[n]], dtype=np.float32))
        for n in W_NAMES}

    in_maps = []
    for ci in range(NCORES):
        sl = slice(ci * BPC, (ci + 1) * BPC)
        im = {"q": q[sl], "k": k[sl], "mask": mask[sl]}
        im.update(ws)
        im.update(bs)
        in_maps.append(im)

    res = bass_utils.run_bass_kernel_spmd(
        nc, in_maps, core_ids=list(range(NCORES)), trace=TRACE)
    LAST_RESULTS = res
    out = np.concatenate([res.results[ci]["out"] for ci in range(NCORES)], axis=0)
    return out


# revision 23
# speedup vs baseline: 1.1412x; 1.1412x over previous
"""Trainium2 Bass kernel for nn_DecoderAttention (B=32, LQ=256, LK=2048, D=512, H=8).

Data-parallel over batch across 8 NeuronCores (4 batch items each).
All matmuls run in bf16 (1 col/cycle at warm 2.4GHz PE clock).

Per batch item (transposed-side layout, contraction always on partitions):
  k loaded via gpsimd cast-DMA (f32 DRAM -> bf16 SBUF), PE-transposed to
  kT[d, l] in l-groups of 512; kp/k2/vp projections stream per group.
  Per head-pair t_: S^T[l, q] for both heads into one [128,512] PSUM bank
  (cols 0:256 / 256:512) -> ONE Exp activation (mask bias per l-partition)
  -> E bf16; U = [v | 1]^T E accumulates over all 16 l-blocks in one PSUM
  bank (even head rows 0:65 cols 0:256 with Z last; odd head rows 63:128
  cols 256:512 with Z first, so ctx rows land lane-aligned at 0:64/64:128).
  Z broadcast via ones-matmul, one reciprocal, two lane-aligned muls,
  + bv as per-partition bias -> ctx pair [128, 256] bf16.
  out_proj = 4-step accumulation over head-pairs; final scores = ncT^T@k2T,
  tanh*CLIP (ACT tanh + DVE mul), mask fill via copy_predicated, DMA out.

Cross-batch overlap via bufs=2 tile pools keeps the PE HAM-warm.
"""
import sys

sys.path.insert(0, "/opt/trn_rl_repo")

import numpy as np

import concourse.bass as bass
import concourse.bacc as bacc
import concourse.mybir as mybir
import concourse.tile as tile
from concourse import bass_utils
from concourse.masks import make_identity

F32 = mybir.dt.float32
BF16 = mybir.dt.bfloat16
U8 = mybir.dt.uint8
AF = mybir.ActivationFunctionType

B, LQ, LK, D, H = 32, 256, 2048, 512, 8
HD = D // H              # 64
NCORES = 8
BPC = B // NCORES        # 4 batch items per core
NLB = LK // 128          # 16 l-blocks
NG = LK // 512           # 4 l-groups
CLIP = 10.0
FLOAT_MIN = -3.4e38
ISQ_HD = 0.125           # 1/sqrt(64)
ISQ_D = float(1.0 / np.sqrt(512.0))
MASK_BIG = -1e30
W_NAMES = ("Wq", "Wk", "Wv", "Wks", "Wo")
B_OF_W = {"Wq": "bq", "Wk": "bk", "Wv": "bv", "Wo": "bo", "Wks": "bks"}

TRACE = False
LAST_RESULTS = None
_CACHE = {}


def _build(reps=1):
    nc = bacc.Bacc("TRN2", target_bir_lowering=False, debug=False)
    q_d = nc.dram_tensor("q", [BPC, LQ, D], F32, kind="ExternalInput").ap()
    k_d = nc.dram_tensor("k", [BPC, LK, D], F32, kind="ExternalInput").ap()
    m_d = nc.dram_tensor("mask", [BPC, LK], U8, kind="ExternalInput").ap()
    w_d = {n: nc.dram_tensor(n, [D, D], F32, kind="ExternalInput").ap()
           for n in W_NAMES}
    b_d = {n: nc.dram_tensor(B_OF_W[n], [D], F32, kind="ExternalInput").ap()
           for n in W_NAMES}
    out_d = nc.dram_tensor("out", [BPC, LQ, LK], F32, kind="ExternalOutput").ap()

    lowp = nc.allow_low_precision("bf16 matmul operands by design")
    lowp.__enter__()
    with tile.TileContext(nc) as tc:
        with (
            tc.tile_pool(name="c1", bufs=1) as c1,          # persistent consts
            tc.tile_pool(name="pb", bufs=2) as pb,          # per-batch persistents
            tc.tile_pool(name="vpap", bufs=24) as vpap,
            tc.tile_pool(name="knp", bufs=5) as knp,        # k natural staging
            tc.tile_pool(name="ktp", bufs=8) as ktp,        # kT group tiles
            tc.tile_pool(name="etp", bufs=3) as etp,        # exp output tiles
            tc.tile_pool(name="thp", bufs=2) as thp,        # final output staging
            tc.tile_pool(name="smal", bufs=2) as smal,      # small working tiles
            tc.tile_pool(name="tr_ps", bufs=1, space="PSUM") as tr_ps,
            tc.tile_pool(name="mm_ps", bufs=2, space="PSUM") as mm_ps,
            tc.tile_pool(name="st_ps", bufs=3, space="PSUM") as st_ps,
            tc.tile_pool(name="u_ps", bufs=2, space="PSUM") as u_ps,
        ):
            # ---------------- one-time setup ----------------
            identf = knp.tile([128, 128], F32, tag="identf", name="identf")
            make_identity(nc, identf)
            ident = c1.tile([128, 128], BF16, tag="ident", name="ident")
            nc.vector.tensor_copy(ident, identf)
            onr = c1.tile([128, 64], BF16, tag="onr", name="onr")
            nc.vector.memset(onr, 1.0)
            fmin = c1.tile([128, 1], F32, tag="fmin", name="fmin")
            nc.vector.memset(fmin, FLOAT_MIN)

            # biases as [128,1] column chunks (f32)
            bcol = {}
            for wn in W_NAMES:
                bn = B_OF_W[wn]
                ap3 = b_d[wn].rearrange("(c p one) -> c p one", p=128, one=1)
                for c in range(4):
                    t = c1.tile([128, 1], F32, tag=f"b_{bn}_{c}", name=f"b_{bn}_{c}")
                    nc.sync.dma_start(out=t, in_=ap3[c])
                    bcol[(bn, c)] = t
            # bv per head [64,1]
            bvh = []
            aph = b_d["Wv"].rearrange("(h p one) -> h p one", p=64, one=1)
            for h in range(H):
                t = c1.tile([64, 1], F32, tag=f"bvh{h}", name=f"bvh{h}")
                nc.sync.dma_start(out=t, in_=aph[h])
                bvh.append(t)

            # transposed weights WT[(wn, dk)] = [128(din chunk), 512(dout)] bf16
            WT = {}
            for wn in ("Wq", "Wk", "Wv", "Wks"):
                wnat = []
                for nj in range(4):
                    t = knp.tile([128, 512], BF16, tag="knat", name="knat")
                    nc.gpsimd.dma_start(
                        out=t, in_=w_d[wn][nj * 128:(nj + 1) * 128, :])
                    wnat.append(t)
                for dk in range(4):
                    pt = tr_ps.tile([128, 512], BF16, tag="tr", name="tr")
                    for nj in range(4):
                        nc.tensor.transpose(
                            pt[:, nj * 128:(nj + 1) * 128],
                            wnat[nj][:, dk * 128:(dk + 1) * 128], ident)
                    wt = c1.tile([128, 512], BF16, tag=f"wt_{wn}_{dk}",
                                 name=f"wt_{wn}_{dk}")
                    nc.any.tensor_copy(wt, pt)
                    WT[(wn, dk)] = wt
            # Wo per head: WoTh[h] = [64(din in head), 512(dout)] bf16, base 0
            WoTh = []
            wnat = []
            for nj in range(4):
                t = knp.tile([128, 512], BF16, tag="knat", name="knat")
                nc.gpsimd.dma_start(
                    out=t, in_=w_d["Wo"][nj * 128:(nj + 1) * 128, :])
                wnat.append(t)
            for h in range(H):
                pt = tr_ps.tile([128, 512], BF16, tag="tr", name="tr")
                for nj in range(4):
                    nc.tensor.transpose(
                        pt[0:64, nj * 128:(nj + 1) * 128],
                        wnat[nj][:, h * 64:(h + 1) * 64], ident)
                wt = c1.tile([64, 512], BF16, tag=f"woth{h}", name=f"woth{h}")
                nc.any.tensor_copy(wt, pt[0:64, :])
                WoTh.append(wt)

            # ---------------- per batch item ----------------
            for bi in [b for _ in range(reps) for b in range(BPC)]:
                # mask: broadcast row to all partitions (uint8) for final fill
                m_row = m_d[bi]
                bcast = bass.AP(tensor=m_row.tensor, offset=m_row.offset,
                                ap=[[0, 128]] + m_row.ap)
                masku8 = pb.tile([128, LK], U8, tag="masku8", name="masku8")
                nc.gpsimd.dma_start(out=masku8, in_=bcast)
                # mask bias columns mb[:, lcg] = -1e30 * mask[lcg*128 + p]
                m16 = smal.tile([16, 128], U8, tag="m16", name="m16")
                nc.sync.dma_start(
                    out=m16, in_=m_row.rearrange("(c p) -> c p", c=16))
                m16f = smal.tile([16, 128], BF16, tag="m16f", name="m16f")
                nc.vector.tensor_copy(m16f, m16)
                mpt = tr_ps.tile([128, 512], BF16, tag="tr", name="tr")
                nc.tensor.transpose(mpt[:, 0:16], m16f, ident[0:16, 0:16])
                mb = pb.tile([128, 16], F32, tag="mb", name="mb")
                nc.vector.tensor_scalar_mul(mb, mpt[:, 0:16], MASK_BIG)

                # q: cast-load + transpose + project
                qnat = []
                for mi in range(2):
                    t = knp.tile([128, 512], BF16, tag="knat", name="knat")
                    nc.gpsimd.dma_start(
                        out=t, in_=q_d[bi, mi * 128:(mi + 1) * 128, :])
                    qnat.append(t)
                qTr = []
                for dk in range(4):
                    pt = tr_ps.tile([128, 512], BF16, tag="tr", name="tr")
                    for mi in range(2):
                        nc.tensor.transpose(
                            pt[:, mi * 128:(mi + 1) * 128],
                            qnat[mi][:, dk * 128:(dk + 1) * 128], ident)
                    t = c1.tile([128, 256], BF16, tag=f"qtr{dk}", name=f"qtr{dk}")
                    nc.any.tensor_copy(t, pt[:, 0:256])
                    qTr.append(t)
                # qpPad[nj]: block-diagonal [128,512]: rows 0:64 cols 0:256 =
                # qp_even, rows 64:128 cols 256:512 = qp_odd, zeros elsewhere.
                # Lets ST run as ONE full p=128 m=512 matmul per l-block.
                qpPad = []
                for nj in range(4):
                    t = pb.tile([128, 512], BF16, tag=f"qpd{nj}", name=f"qpd{nj}")
                    nc.vector.memset(t, 0.0)
                    qpPad.append(t)
                for nj in range(4):
                    pt = mm_ps.tile([128, 512], F32, tag="mm", name="mm")
                    for dk in range(4):
                        nc.tensor.matmul(
                            pt[:, 0:256], WT[("Wq", dk)][:, nj * 128:(nj + 1) * 128],
                            qTr[dk], start=(dk == 0), stop=(dk == 3))
                    nc.scalar.activation(
                        qpPad[nj][0:64, 0:256], pt[0:64, 0:256], AF.Identity,
                        bias=bcol[("bq", nj)][0:64, :], scale=1.0)
                    nc.scalar.activation(
                        qpPad[nj][64:128, 256:512], pt[64:128, 0:256], AF.Identity,
                        bias=bcol[("bq", nj)][64:128, :], scale=1.0)

                kpTr = [pb.tile([128, LK], BF16, tag=f"kpt{nj}", name=f"kpt{nj}")
                        for nj in range(4)]
                k2Tr = [pb.tile([128, LK], BF16, tag=f"k2t{nj}", name=f"k2t{nj}")
                        for nj in range(4)]
                vpa = []

                for g in range(NG):
                    lbase = g * 512
                    knat = []
                    for li in range(4):
                        t = knp.tile([128, 512], BF16, tag="knat", name="knat")
                        nc.gpsimd.dma_start(
                            out=t, in_=k_d[bi, lbase + li * 128:lbase + (li + 1) * 128, :])
                        knat.append(t)
                    kTg = []
                    for dk in range(4):
                        pt = tr_ps.tile([128, 512], BF16, tag="tr", name="tr")
                        for li in range(4):
                            nc.tensor.transpose(
                                pt[:, li * 128:(li + 1) * 128],
                                knat[li][:, dk * 128:(dk + 1) * 128], ident)
                        t = ktp.tile([128, 512], BF16, tag="ktg", name="ktg")
                        if dk % 2 == 0:
                            nc.vector.tensor_copy(t, pt)
                        else:
                            nc.scalar.activation(t, pt, AF.Identity,
                                                 bias=0.0, scale=1.0)
                        kTg.append(t)
                    # kp projection (bias via DVE), k2 (bias via ACT)
                    for nj in range(4):
                        pt = mm_ps.tile([128, 512], F32, tag="mm", name="mm")
                        for dk in range(4):
                            nc.tensor.matmul(
                                pt, WT[("Wk", dk)][:, nj * 128:(nj + 1) * 128],
                                kTg[dk], start=(dk == 0), stop=(dk == 3))
                        nc.vector.tensor_scalar_add(
                            out=kpTr[nj][:, lbase:lbase + 512],
                            in0=pt, scalar1=bcol[("bk", nj)][:, :])
                    for nj in range(4):
                        pt = mm_ps.tile([128, 512], F32, tag="mm", name="mm")
                        for dk in range(4):
                            nc.tensor.matmul(
                                pt, WT[("Wks", dk)][:, nj * 128:(nj + 1) * 128],
                                kTg[dk], start=(dk == 0), stop=(dk == 3))
                        nc.scalar.activation(
                            k2Tr[nj][:, lbase:lbase + 512], pt, AF.Identity,
                            bias=bcol[("bks", nj)][:, :], scale=1.0)
                    # vp natural [l, dout] -> vpa per head: [v(64) | one]
                    for lb in range(4):
                        pt = mm_ps.tile([128, 512], F32, tag="mm", name="mm")
                        for dk in range(4):
                            nc.tensor.matmul(
                                pt, kTg[dk][:, lb * 128:(lb + 1) * 128],
                                WT[("Wv", dk)], start=(dk == 0), stop=(dk == 3))
                        vt = vpap.tile([128, H * 65], BF16, tag="vpa", name="vpa")
                        vt3 = vt.rearrange("p (h c) -> p h c", c=65)
                        nc.vector.tensor_copy(
                            vt3[:, :, 0:64],
                            pt.rearrange("p (h c) -> p h c", c=64))
                        nc.vector.memset(vt3[:, :, 64:65], 1.0)
                        vpa.append(vt)

                # attention per head pair
                ctxh = []
                for t_ in range(4):
                    u = u_ps.tile([128, 512], F32, tag="u", name="u")
                    for lc in range(NLB):
                        sp = st_ps.tile([128, 512], F32, tag="st", name="st")
                        nc.tensor.matmul(
                            sp, kpTr[t_][:, lc * 128:(lc + 1) * 128],
                            qpPad[t_], start=True, stop=True)
                        et = etp.tile([128, 512], BF16, tag="et", name="et")
                        nc.scalar.activation(
                            et, sp, AF.Exp,
                            bias=mb[:, lc:lc + 1], scale=ISQ_HD)
                        nc.tensor.matmul(
                            u[0:65, 0:256],
                            vpa[lc][:, (2 * t_) * 65:(2 * t_) * 65 + 65],
                            et[:, 0:256], start=(lc == 0), stop=(lc == NLB - 1),
                            skip_group_check=True)
                        # start only on the bank's first matmul: start=True
                        # clears the WHOLE bank; the odd head's first matmul
                        # relies on has_written=0 -> overwrite semantics.
                        nc.tensor.matmul(
                            u[0:65, 256:512],
                            vpa[lc][:, (2 * t_ + 1) * 65:(2 * t_ + 1) * 65 + 65],
                            et[:, 256:512], start=False, stop=(lc == NLB - 1),
                            skip_group_check=True)
                    # Z row -> bf16, broadcast via ones-matmul, divide, + bv
                    zr = smal.tile([128, 512], BF16, tag="zr", name="zr")
                    nc.vector.tensor_copy(zr[64:65, :], u[64:65, :])
                    zb = mm_ps.tile([128, 512], F32, tag="mm", name="mm")
                    nc.tensor.matmul(zb[0:64, :], onr[64:65, :],
                                     zr[64:65, :], start=True, stop=True)
                    rz = smal.tile([64, 512], F32, tag="rz", name="rz")
                    nc.vector.reciprocal(rz, zb[0:64, :])
                    ct = c1.tile([64, 512], BF16, tag=f"ctx{t_}", name=f"ctx{t_}")
                    nc.vector.tensor_mul(ct, u[0:64, :], rz)
                    nc.vector.tensor_scalar_add(
                        out=ct[:, 0:256], in0=ct[:, 0:256],
                        scalar1=bvh[2 * t_][:, :])
                    nc.vector.tensor_scalar_add(
                        out=ct[:, 256:512], in0=ct[:, 256:512],
                        scalar1=bvh[2 * t_ + 1][:, :])
                    ctxh.append(ct)

                # out_proj: ncT[dout, q] = sum over heads (p=64 each)
                ncTr = []
                for nj in range(4):
                    pt = mm_ps.tile([128, 512], F32, tag="mm", name="mm")
                    for t_ in range(4):
                        for hh in range(2):
                            nc.tensor.matmul(
                                pt[:, 0:256],
                                WoTh[2 * t_ + hh][:, nj * 128:(nj + 1) * 128],
                                ctxh[t_][:, hh * 256:(hh + 1) * 256],
                                start=(t_ == 0 and hh == 0),
                                stop=(t_ == 3 and hh == 1))
                    t = c1.tile([128, 256], BF16, tag=f"nct{nj}", name=f"nct{nj}")
                    nc.scalar.activation(t, pt[:, 0:256], AF.Identity,
                                         bias=bcol[("bo", nj)][:, :], scale=1.0)
                    ncTr.append(t)

                # final scores + tanh clip + mask fill
                for mi in range(2):
                    for lg in range(4):
                        pt = mm_ps.tile([128, 512], F32, tag="mm", name="mm")
                        for nk in range(4):
                            nc.tensor.matmul(
                                pt, ncTr[nk][:, mi * 128:(mi + 1) * 128],
                                k2Tr[nk][:, lg * 512:(lg + 1) * 512],
                                start=(nk == 0), stop=(nk == 3))
                        th = thp.tile([128, 512], F32, tag="th", name="th")
                        nc.scalar.activation(th, pt, AF.Tanh,
                                             bias=0.0, scale=ISQ_D)
                        nc.vector.tensor_scalar_mul(th, th, CLIP)
                        nc.vector.copy_predicated(
                            th, masku8[:, lg * 512:(lg + 1) * 512],
                            fmin.to_broadcast([128, 512]))
                        nc.sync.dma_start(
                            out=out_d[bi, mi * 128:(mi + 1) * 128,
                                      lg * 512:(lg + 1) * 512],
                            in_=th)
    lowp.__exit__(None, None, None)
    nc.finalize()
    return nc


def kernel(**inputs):
    global LAST_RESULTS
    import os
    reps = int(os.environ.get("KERNEL_REPS", "1"))
    key = ("nc", reps)
    if key not in _CACHE:
        _CACHE[key] = _build(reps)
    nc = _CACHE[key]

    q = np.ascontiguousarray(np.asarray(inputs["q"], dtype=np.float32))
    k = np.ascontiguousarray(np.asarray(inputs["k"], dtype=np.float32))
    mask = np.ascontiguousarray(np.asarray(inputs["mask"]).astype(np.uint8))
    ws = {n: np.ascontiguousarray(np.asarray(inputs[n], dtype=np.float32))
          for n in W_NAMES}
    bs = {B_OF_W[n]: np.ascontiguousarray(
        np.asarray(inputs[B_OF_W[n]], dtype=np.float32))
        for n in W_NAMES}

    in_maps = []
    for ci in range(NCORES):
        sl = slice(ci * BPC, (ci + 1) * BPC)
        im = {"q": q[sl], "k": k[sl], "mask": mask[sl]}
        im.update(ws)
        im.update(bs)
        in_maps.append(im)

    res = bass_utils.run_bass_kernel_spmd(
        nc, in_maps, core_ids=list(range(NCORES)), trace=TRACE)
    LAST_RESULTS = res
    out = np.concatenate([res.results[ci]["out"] for ci in range(NCORES)], axis=0)
    return out


# revision 24
# speedup vs baseline: 1.1480x; 1.0059x over previous
"""Trainium2 Bass kernel for nn_DecoderAttention (B=32, LQ=256, LK=2048, D=512, H=8).

Data-parallel over batch across 8 NeuronCores (4 batch items each).
All matmuls run in bf16 (1 col/cycle at warm 2.4GHz PE clock).

Per batch item (transposed-side layout, contraction always on partitions):
  k loaded via gpsimd cast-DMA (f32 DRAM -> bf16 SBUF), PE-transposed to
  kT[d, l] in l-groups of 512; kp/k2/vp projections stream per group.
  Per head-pair t_: S^T[l, q] for both heads into one [128,512] PSUM bank
  (cols 0:256 / 256:512) -> ONE Exp activation (mask bias per l-partition)
  -> E bf16; U = [v | 1]^T E accumulates over all 16 l-blocks in one PSUM
  bank (even head rows 0:65 cols 0:256 with Z last; odd head rows 63:128
  cols 256:512 with Z first, so ctx rows land lane-aligned at 0:64/64:128).
  Z broadcast via ones-matmul, one reciprocal, two lane-aligned muls,
  + bv as per-partition bias -> ctx pair [128, 256] bf16.
  out_proj = 4-step accumulation over head-pairs; final scores = ncT^T@k2T,
  tanh*CLIP (ACT tanh + DVE mul), mask fill via copy_predicated, DMA out.

Cross-batch overlap via bufs=2 tile pools keeps the PE HAM-warm.
"""
import sys

sys.path.insert(0, "/opt/trn_rl_repo")

import numpy as np

import concourse.bass as bass
import concourse.bacc as bacc
import concourse.mybir as mybir
import concourse.tile as tile
from concourse import bass_utils
from concourse.masks import make_identity

F32 = mybir.dt.float32
BF16 = mybir.dt.bfloat16
U8 = mybir.dt.uint8
AF = mybir.ActivationFunctionType

B, LQ, LK, D, H = 32, 256, 2048, 512, 8
HD = D // H              # 64
NCORES = 8
BPC = B // NCORES        # 4 batch items per core
NLB = LK // 128          # 16 l-blocks
NG = LK // 512           # 4 l-groups
CLIP = 10.0
FLOAT_MIN = -3.4e38
ISQ_HD = 0.125           # 1/sqrt(64)
ISQ_D = float(1.0 / np.sqrt(512.0))
MASK_BIG = -1e30
W_NAMES = ("Wq", "Wk", "Wv", "Wks", "Wo")
B_OF_W = {"Wq": "bq", "Wk": "bk", "Wv": "bv", "Wo": "bo", "Wks": "bks"}

TRACE = False
LAST_RESULTS = None
_CACHE = {}


def _build(reps=1):
    nc = bacc.Bacc("TRN2", target_bir_lowering=False, debug=False)
    q_d = nc.dram_tensor("q", [BPC, LQ, D], F32, kind="ExternalInput").ap()
    k_d = nc.dram_tensor("k", [BPC, LK, D], F32, kind="ExternalInput").ap()
    m_d = nc.dram_tensor("mask", [BPC, LK], U8, kind="ExternalInput").ap()
    w_d = {n: nc.dram_tensor(n, [D, D], F32, kind="ExternalInput").ap()
           for n in W_NAMES}
    b_d = {n: nc.dram_tensor(B_OF_W[n], [D], F32, kind="ExternalInput").ap()
           for n in W_NAMES}
    out_d = nc.dram_tensor("out", [BPC, LQ, LK], F32, kind="ExternalOutput").ap()

    lowp = nc.allow_low_precision("bf16 matmul operands by design")
    lowp.__enter__()
    with tile.TileContext(nc) as tc:
        with (
            tc.tile_pool(name="c1", bufs=1) as c1,          # persistent consts
            tc.tile_pool(name="pb", bufs=2) as pb,          # per-batch persistents
            tc.tile_pool(name="vpap", bufs=24) as vpap,
            tc.tile_pool(name="knp", bufs=5) as knp,        # k natural staging
            tc.tile_pool(name="ktp", bufs=8) as ktp,        # kT group tiles
            tc.tile_pool(name="etp", bufs=4) as etp,        # exp output tiles
            tc.tile_pool(name="thp", bufs=3) as thp,        # final output staging
            tc.tile_pool(name="smal", bufs=2) as smal,      # small working tiles
            tc.tile_pool(name="tr_ps", bufs=1, space="PSUM") as tr_ps,
            tc.tile_pool(name="mm_ps", bufs=3, space="PSUM") as mm_ps,
            tc.tile_pool(name="st_ps", bufs=2, space="PSUM") as st_ps,
            tc.tile_pool(name="u_ps", bufs=2, space="PSUM") as u_ps,
        ):
            # ---------------- one-time setup ----------------
            identf = knp.tile([128, 128], F32, tag="identf", name="identf")
            make_identity(nc, identf)
            ident = c1.tile([128, 128], BF16, tag="ident", name="ident")
            nc.vector.tensor_copy(ident, identf)
            # HAM warm-up: ~20 back-to-back full-width matmuls unthrottle
            # the PE clock (1.2 -> 2.4 GHz) while the first DMAs land.
            wscr = c1.tile([128, 512], BF16, tag="wscr", name="wscr")
            nc.vector.memset(wscr, 1.0)
            for _wi in range(20):
                wp = mm_ps.tile([128, 512], F32, tag="mm", name="mm")
                nc.tensor.matmul(wp, ident, wscr, start=True, stop=True)
            onr = c1.tile([128, 64], BF16, tag="onr", name="onr")
            nc.vector.memset(onr, 1.0)
            fmin = c1.tile([128, 1], F32, tag="fmin", name="fmin")
            nc.vector.memset(fmin, FLOAT_MIN)

            # biases as [128,1] column chunks (f32)
            bcol = {}
            for wn in W_NAMES:
                bn = B_OF_W[wn]
                ap3 = b_d[wn].rearrange("(c p one) -> c p one", p=128, one=1)
                for c in range(4):
                    t = c1.tile([128, 1], F32, tag=f"b_{bn}_{c}", name=f"b_{bn}_{c}")
                    nc.sync.dma_start(out=t, in_=ap3[c])
                    bcol[(bn, c)] = t
            # bv per head [64,1]
            bvh = []
            aph = b_d["Wv"].rearrange("(h p one) -> h p one", p=64, one=1)
            for h in range(H):
                t = c1.tile([64, 1], F32, tag=f"bvh{h}", name=f"bvh{h}")
                nc.sync.dma_start(out=t, in_=aph[h])
                bvh.append(t)

            # transposed weights WT[(wn, dk)] = [128(din chunk), 512(dout)] bf16
            WT = {}
            for wn in ("Wq", "Wk", "Wv", "Wks"):
                wnat = []
                for nj in range(4):
                    tf = knp.tile([128, 512], F32, tag="wnf", name="wnf")
                    nc.sync.dma_start(
                        out=tf, in_=w_d[wn][nj * 128:(nj + 1) * 128, :])
                    t = knp.tile([128, 512], BF16, tag="knat", name="knat")
                    nc.vector.tensor_copy(t, tf)
                    wnat.append(t)
                for dk in range(4):
                    pt = tr_ps.tile([128, 512], BF16, tag="tr", name="tr")
                    for nj in range(4):
                        nc.tensor.transpose(
                            pt[:, nj * 128:(nj + 1) * 128],
                            wnat[nj][:, dk * 128:(dk + 1) * 128], ident)
                    wt = c1.tile([128, 512], BF16, tag=f"wt_{wn}_{dk}",
                                 name=f"wt_{wn}_{dk}")
                    nc.any.tensor_copy(wt, pt)
                    WT[(wn, dk)] = wt
            # Wo per head: WoTh[h] = [64(din in head), 512(dout)] bf16, base 0
            WoTh = []
            wnat = []
            for nj in range(4):
                tf = knp.tile([128, 512], F32, tag="wnf", name="wnf")
                nc.sync.dma_start(
                    out=tf, in_=w_d["Wo"][nj * 128:(nj + 1) * 128, :])
                t = knp.tile([128, 512], BF16, tag="knat", name="knat")
                nc.vector.tensor_copy(t, tf)
                wnat.append(t)
            for h in range(H):
                pt = tr_ps.tile([128, 512], BF16, tag="tr", name="tr")
                for nj in range(4):
                    nc.tensor.transpose(
                        pt[0:64, nj * 128:(nj + 1) * 128],
                        wnat[nj][:, h * 64:(h + 1) * 64], ident)
                wt = c1.tile([64, 512], BF16, tag=f"woth{h}", name=f"woth{h}")
                nc.any.tensor_copy(wt, pt[0:64, :])
                WoTh.append(wt)

            # ---------------- per batch item ----------------
            for bi in [b for _ in range(reps) for b in range(BPC)]:
                # mask: broadcast row to all partitions (uint8) for final fill
                m_row = m_d[bi]
                bcast = bass.AP(tensor=m_row.tensor, offset=m_row.offset,
                                ap=[[0, 128]] + m_row.ap)
                masku8 = pb.tile([128, LK], U8, tag="masku8", name="masku8")
                nc.gpsimd.dma_start(out=masku8, in_=bcast)
                # mask bias columns mb[:, lcg] = -1e30 * mask[lcg*128 + p]
                m16 = smal.tile([16, 128], U8, tag="m16", name="m16")
                nc.sync.dma_start(
                    out=m16, in_=m_row.rearrange("(c p) -> c p", c=16))
                m16f = smal.tile([16, 128], BF16, tag="m16f", name="m16f")
                nc.vector.tensor_copy(m16f, m16)
                mpt = tr_ps.tile([128, 512], BF16, tag="tr", name="tr")
                nc.tensor.transpose(mpt[:, 0:16], m16f, ident[0:16, 0:16])
                mb = pb.tile([128, 16], F32, tag="mb", name="mb")
                nc.vector.tensor_scalar_mul(mb, mpt[:, 0:16], MASK_BIG)

                # q: cast-load + transpose + project
                qnat = []
                for mi in range(2):
                    t = knp.tile([128, 512], BF16, tag="knat", name="knat")
                    nc.gpsimd.dma_start(
                        out=t, in_=q_d[bi, mi * 128:(mi + 1) * 128, :])
                    qnat.append(t)
                qTr = []
                for dk in range(4):
                    pt = tr_ps.tile([128, 512], BF16, tag="tr", name="tr")
                    for mi in range(2):
                        nc.tensor.transpose(
                            pt[:, mi * 128:(mi + 1) * 128],
                            qnat[mi][:, dk * 128:(dk + 1) * 128], ident)
                    t = c1.tile([128, 256], BF16, tag=f"qtr{dk}", name=f"qtr{dk}")
                    nc.any.tensor_copy(t, pt[:, 0:256])
                    qTr.append(t)
                # qpPad[nj]: block-diagonal [128,512]: rows 0:64 cols 0:256 =
                # qp_even, rows 64:128 cols 256:512 = qp_odd, zeros elsewhere.
                # Lets ST run as ONE full p=128 m=512 matmul per l-block.
                qpPad = []
                for nj in range(4):
                    t = pb.tile([128, 512], BF16, tag=f"qpd{nj}", name=f"qpd{nj}")
                    nc.vector.memset(t, 0.0)
                    qpPad.append(t)
                for nj in range(4):
                    pt = mm_ps.tile([128, 512], F32, tag="mm", name="mm")
                    for dk in range(4):
                        nc.tensor.matmul(
                            pt[:, 0:256], WT[("Wq", dk)][:, nj * 128:(nj + 1) * 128],
                            qTr[dk], start=(dk == 0), stop=(dk == 3))
                    nc.scalar.activation(
                        qpPad[nj][0:64, 0:256], pt[0:64, 0:256], AF.Identity,
                        bias=bcol[("bq", nj)][0:64, :], scale=1.0)
                    nc.scalar.activation(
                        qpPad[nj][64:128, 256:512], pt[64:128, 0:256], AF.Identity,
                        bias=bcol[("bq", nj)][64:128, :], scale=1.0)

                kpTr = [pb.tile([128, LK], BF16, tag=f"kpt{nj}", name=f"kpt{nj}")
                        for nj in range(4)]
                k2Tr = [pb.tile([128, LK], BF16, tag=f"k2t{nj}", name=f"k2t{nj}")
                        for nj in range(4)]
                vpa = []

                for g in range(NG):
                    lbase = g * 512
                    knat = []
                    for li in range(4):
                        t = knp.tile([128, 512], BF16, tag="knat", name="knat")
                        nc.gpsimd.dma_start(
                            out=t, in_=k_d[bi, lbase + li * 128:lbase + (li + 1) * 128, :])
                        knat.append(t)
                    kTg = []
                    for dk in range(4):
                        pt = tr_ps.tile([128, 512], BF16, tag="tr", name="tr")
                        for li in range(4):
                            nc.tensor.transpose(
                                pt[:, li * 128:(li + 1) * 128],
                                knat[li][:, dk * 128:(dk + 1) * 128], ident)
                        t = ktp.tile([128, 512], BF16, tag="ktg", name="ktg")
                        if dk % 2 == 0:
                            nc.vector.tensor_copy(t, pt)
                        else:
                            nc.scalar.activation(t, pt, AF.Identity,
                                                 bias=0.0, scale=1.0)
                        kTg.append(t)
                    # kp projection (bias via DVE), k2 (bias via ACT)
                    for nj in range(4):
                        pt = mm_ps.tile([128, 512], F32, tag="mm", name="mm")
                        for dk in range(4):
                            nc.tensor.matmul(
                                pt, WT[("Wk", dk)][:, nj * 128:(nj + 1) * 128],
                                kTg[dk], start=(dk == 0), stop=(dk == 3))
                        nc.vector.tensor_scalar_add(
                            out=kpTr[nj][:, lbase:lbase + 512],
                            in0=pt, scalar1=bcol[("bk", nj)][:, :])
                    for nj in range(4):
                        pt = mm_ps.tile([128, 512], F32, tag="mm", name="mm")
                        for dk in range(4):
                            nc.tensor.matmul(
                                pt, WT[("Wks", dk)][:, nj * 128:(nj + 1) * 128],
                                kTg[dk], start=(dk == 0), stop=(dk == 3))
                        nc.scalar.activation(
                            k2Tr[nj][:, lbase:lbase + 512], pt, AF.Identity,
                            bias=bcol[("bks", nj)][:, :], scale=1.0)
                    # vp natural [l, dout] -> vpa per head: [v(64) | one]
                    for lb in range(4):
                        pt = mm_ps.tile([128, 512], F32, tag="mm", name="mm")
                        for dk in range(4):
                            nc.tensor.matmul(
                                pt, kTg[dk][:, lb * 128:(lb + 1) * 128],
                                WT[("Wv", dk)], start=(dk == 0), stop=(dk == 3))
                        vt = vpap.tile([128, H * 65], BF16, tag="vpa", name="vpa")
                        vt3 = vt.rearrange("p (h c) -> p h c", c=65)
                        nc.vector.tensor_copy(
                            vt3[:, :, 0:64],
                            pt.rearrange("p (h c) -> p h c", c=64))
                        nc.vector.memset(vt3[:, :, 64:65], 1.0)
                        vpa.append(vt)

                # attention per head pair
                ctxh = []
                for t_ in range(4):
                    u = u_ps.tile([128, 512], F32, tag="u", name="u")
                    for lc in range(NLB):
                        sp = st_ps.tile([128, 512], F32, tag="st", name="st")
                        nc.tensor.matmul(
                            sp, kpTr[t_][:, lc * 128:(lc + 1) * 128],
                            qpPad[t_], start=True, stop=True)
                        et = etp.tile([128, 512], BF16, tag="et", name="et")
                        nc.scalar.activation(
                            et, sp, AF.Exp,
                            bias=mb[:, lc:lc + 1], scale=ISQ_HD)
                        nc.tensor.matmul(
                            u[0:65, 0:256],
                            vpa[lc][:, (2 * t_) * 65:(2 * t_) * 65 + 65],
                            et[:, 0:256], start=(lc == 0), stop=(lc == NLB - 1),
                            skip_group_check=True)
                        # start only on the bank's first matmul: start=True
                        # clears the WHOLE bank; the odd head's first matmul
                        # relies on has_written=0 -> overwrite semantics.
                        nc.tensor.matmul(
                            u[0:65, 256:512],
                            vpa[lc][:, (2 * t_ + 1) * 65:(2 * t_ + 1) * 65 + 65],
                            et[:, 256:512], start=False, stop=(lc == NLB - 1),
                            skip_group_check=True)
                    # Z row -> bf16, broadcast via ones-matmul, divide, + bv
                    zr = smal.tile([128, 512], BF16, tag="zr", name="zr")
                    nc.vector.tensor_copy(zr[64:65, :], u[64:65, :])
                    zb = mm_ps.tile([128, 512], F32, tag="mm", name="mm")
                    nc.tensor.matmul(zb[0:64, :], onr[64:65, :],
                                     zr[64:65, :], start=True, stop=True)
                    rz = smal.tile([64, 512], F32, tag="rz", name="rz")
                    nc.vector.reciprocal(rz, zb[0:64, :])
                    ct = c1.tile([64, 512], BF16, tag=f"ctx{t_}", name=f"ctx{t_}")
                    nc.vector.tensor_mul(ct, u[0:64, :], rz)
                    nc.vector.tensor_scalar_add(
                        out=ct[:, 0:256], in0=ct[:, 0:256],
                        scalar1=bvh[2 * t_][:, :])
                    nc.vector.tensor_scalar_add(
                        out=ct[:, 256:512], in0=ct[:, 256:512],
                        scalar1=bvh[2 * t_ + 1][:, :])
                    ctxh.append(ct)

                # out_proj: ncT[dout, q] = sum over heads (p=64 each)
                ncTr = []
                for nj in range(4):
                    pt = mm_ps.tile([128, 512], F32, tag="mm", name="mm")
                    for t_ in range(4):
                        for hh in range(2):
                            nc.tensor.matmul(
                                pt[:, 0:256],
                                WoTh[2 * t_ + hh][:, nj * 128:(nj + 1) * 128],
                                ctxh[t_][:, hh * 256:(hh + 1) * 256],
                                start=(t_ == 0 and hh == 0),
                                stop=(t_ == 3 and hh == 1))
                    t = c1.tile([128, 256], BF16, tag=f"nct{nj}", name=f"nct{nj}")
                    nc.scalar.activation(t, pt[:, 0:256], AF.Identity,
                                         bias=bcol[("bo", nj)][:, :], scale=1.0)
                    ncTr.append(t)

                # final scores + tanh clip + mask fill
                for mi in range(2):
                    for lg in range(4):
                        pt = mm_ps.tile([128, 512], F32, tag="mm", name="mm")
                        for nk in range(4):
                            nc.tensor.matmul(
                                pt, ncTr[nk][:, mi * 128:(mi + 1) * 128],
                                k2Tr[nk][:, lg * 512:(lg + 1) * 512],
                                start=(nk == 0), stop=(nk == 3))
                        th = thp.tile([128, 512], F32, tag="th", name="th")
                        nc.scalar.activation(th, pt, AF.Tanh,
                                             bias=0.0, scale=ISQ_D)
                        nc.vector.tensor_scalar_mul(th, th, CLIP)
                        nc.vector.copy_predicated(
                            th, masku8[:, lg * 512:(lg + 1) * 512],
                            fmin.to_broadcast([128, 512]))
                        nc.sync.dma_start(
                            out=out_d[bi, mi * 128:(mi + 1) * 128,
                                      lg * 512:(lg + 1) * 512],
                            in_=th)
    lowp.__exit__(None, None, None)
    nc.finalize()
    return nc


def kernel(**inputs):
    global LAST_RESULTS
    import os
    reps = int(os.environ.get("KERNEL_REPS", "1"))
    key = ("nc", reps)
    if key not in _CACHE:
        _CACHE[key] = _build(reps)
    nc = _CACHE[key]

    q = np.ascontiguousarray(np.asarray(inputs["q"], dtype=np.float32))
    k = np.ascontiguousarray(np.asarray(inputs["k"], dtype=np.float32))
    mask = np.ascontiguousarray(np.asarray(inputs["mask"]).astype(np.uint8))
    ws = {n: np.ascontiguousarray(np.asarray(inputs[n], dtype=np.float32))
          for n in W_NAMES}
    bs = {B_OF_W[n]: np.ascontiguousarray(
        np.asarray(inputs[B_OF_W[n]], dtype=np.float32))
        for n in W_NAMES}

    in_maps = []
    for ci in range(NCORES):
        sl = slice(ci * BPC, (ci + 1) * BPC)
        im = {"q": q[sl], "k": k[sl], "mask": mask[sl]}
        im.update(ws)
        im.update(bs)
        in_maps.append(im)

    res = bass_utils.run_bass_kernel_spmd(
        nc, in_maps, core_ids=list(range(NCORES)), trace=TRACE)
    LAST_RESULTS = res
    out = np.concatenate([res.results[ci]["out"] for ci in range(NCORES)], axis=0)
    return out
